# revision 1
# baseline (speedup 1.0000x reference)
"""BERT-CRF NER Viterbi decode kernel for Trainium2 (8 NeuronCores).

Strategy (data-parallel over batch, 8 rows/core), raw Bass (no Tile):
  - host: shard hidden_states [64,512,768] -> 8 x [8,512,768], pre-transpose to
    [8,768,512] so the PE matmul needs no on-device transpose; fold bias b into
    the transition matrix (feat enters the max additively per 'to').
  - device (per core):
      feats = W.T @ hsT per batch row -> PSUM [9,512] (6 K-chunks of 128)
      ACT copies PSUM->SBUF, DMA spreads to [(32*tc+b) partition, (to,tl)]
      transfeat[t,to,from] = trans[to,from]+b[to]+feat[t,to] (one bulk DVE op)
      Viterbi forward scan, t=1..511: 2 DVE ops per step on [8, 9x9]:
        scores = transfeat[t] + delta[t-1] (broadcast over 'to')
        delta[t] = reduce_max over 'from'   (stored for all t)
      bulk psi: argmax_from(trans[to,from]+delta[t-1,from]) for all t at once
        (is_ge/iota-encode/reduce trick; first-tie wins, matching jnp.argmax)
  - host: gather, backtrace (trivial pointer chase), return [64,512] int32.
"""

import numpy as np
from contextlib import ExitStack

import concourse.bass as bass
from concourse import mybir
from concourse.bass_utils import run_bass_kernel_spmd

B, T, H, L = 64, 512, 768, 9
NC = 8              # cores
BL = B // NC        # batch rows per core = 8
KC = H // 128       # 6 contraction chunks
TC = 4              # t-chunks of 128 for the spread layout
TL = T // TC        # 128
START = 7
NEG = -10000.0

F32 = mybir.dt.float32
ADD = mybir.AluOpType.add
MAX = mybir.AluOpType.max
GE = mybir.AluOpType.is_ge
MUL = mybir.AluOpType.mult
AXX = mybir.AxisListType.X


LC = 8          # compact 'to' labels: (0..6, 8); START row dropped
FC = 7          # compact 'from' labels: 0..6
LAB = [0, 1, 2, 3, 4, 5, 6, 8]


def build_program():
    nc = bass.Bass("TRN2", target_bir_lowering=False,
                   detect_race_conditions=False)

    hsT_d = nc.dram_tensor("hsT", [BL, H, T], F32, kind="ExternalInput")
    wk_d = nc.dram_tensor("wk", [128, KC * LC], F32, kind="ExternalInput")
    # trans (+bias) compact [to8', from7'] replicated; d7c = trans[to',7]
    trep_d = nc.dram_tensor("trep", [128, LC * FC], F32, kind="ExternalInput")
    iot_d = nc.dram_tensor("iot", [128, FC * FC], mybir.dt.bfloat16,
                           kind="ExternalInput")
    d7c_d = nc.dram_tensor("d7c", [BL, LC], F32, kind="ExternalInput")
    psiv_d = nc.dram_tensor("psiv", [TC * BL, TL * FC], F32,
                            kind="ExternalOutput")
    dfin_d = nc.dram_tensor("dfin", [BL, 2 * LC], F32, kind="ExternalOutput")

    NB = 4                                   # ht load buffers

    with ExitStack() as ctx:
        def sb(name, shape):
            return ctx.enter_context(nc.sbuf_tensor(name, shape, F32))
        wk = sb("wk_sb", [128, KC * LC])
        trep = sb("trep_sb", [128, LC * FC])
        iot = ctx.enter_context(nc.sbuf_tensor("iot_sb", [128, FC * FC],
                                               mybir.dt.bfloat16))
        d7c = sb("d7c_sb", [BL, LC])
        # delta history, chunk-local: rows [32*tc .. 32*tc+8) slot j holds
        # delta_{128*tc + j - 1} (compact LC labels); slot TL is outgoing
        delta_all = sb("delta_all", [128, (TL + 1) * LC])
        feats_sp = sb("feats_sp", [128, LC * TL])
        mx = sb("mx", [128, TL * FC])
        psiv = sb("psiv_sb", [128, TL * FC])
        sc = sb("sc", [128, LC * FC])
        tf = sb("tf", [128, TL * LC * FC])
        tf_sem = ctx.enter_context(nc.semaphore("tf_sem"))
        sca = sb("sca", [128, TL * FC * FC])
        eq = ctx.enter_context(nc.sbuf_tensor("eq", [128, TL * FC * FC],
                                              mybir.dt.bfloat16))
        msk = ctx.enter_context(nc.sbuf_tensor("msk", [128, TL * FC * FC],
                                               mybir.dt.bfloat16))
        ht = [sb(f"ht{i}", [128, KC * T]) for i in range(NB)]
        stage = sb("stage", [LC, BL * T])
        psum = [ctx.enter_context(nc.psum_tensor(f"psum{b}", [LC, T], F32))
                for b in range(BL)]

        in_sem = ctx.enter_context(nc.semaphore("in_sem"))
        hs_sems = [ctx.enter_context(nc.semaphore(f"hs_sem{i}"))
                   for i in range(NB)]
        pe_sem = ctx.enter_context(nc.semaphore("pe_sem"))
        cp_sem = ctx.enter_context(nc.semaphore("cp_sem"))
        sp_sem = ctx.enter_context(nc.semaphore("sp_sem"))
        ms_sem = ctx.enter_context(nc.semaphore("ms_sem"))
        dv_sem = ctx.enter_context(nc.semaphore("dv_sem"))
        bn_sem = ctx.enter_context(nc.semaphore("bn_sem"))
        bn2_sem = ctx.enter_context(nc.semaphore("bn2_sem"))
        out_sem = ctx.enter_context(nc.semaphore("out_sem"))
        block = ctx.enter_context(nc.Block())

        def rep4(t_sb, a, c):
            # [128, a*c] const -> [128, TL(bcast), a, c]
            return (t_sb[:, :].rearrange("p (a f) -> p a f", f=c)
                    .unsqueeze(1).broadcast_to([128, TL, a, c]))

        @block.gpsimd
        def _(g):
            g.memset(feats_sp[:, :], 0.0)
            g.memset(delta_all[:, :], 0.0).then_inc(ms_sem, 1)

        @block.sync
        def _(sync):
            sync.dma_start(wk[:, :], wk_d[:, :]).then_inc(in_sem, 16)
            sync.dma_start(trep[:, :], trep_d[:, :]).then_inc(in_sem, 16)
            sync.dma_start(iot[:, :], iot_d[:, :]).then_inc(in_sem, 16)
            sync.dma_start(d7c[:, :], d7c_d[:, :]).then_inc(in_sem, 16)
            for b in range(BL):
                src = hsT_d[b, :, :].rearrange("(kc p) t -> p kc t", p=128)
                dst = ht[b % NB][:, :].rearrange("p (kc t) -> p kc t", kc=KC)
                if b >= NB:   # buffer free when b-NB's matmuls done
                    sync.wait_ge(pe_sem, b - NB + 1)
                sync.dma_start(dst, src).then_inc(hs_sems[b % NB], 16)
            sync.wait_ge(ms_sem, 1)
            # spread feats (stage -> feats_sp), per b after its ACT copy
            for b in range(BL):
                sync.wait_ge(cp_sem, b + 1)
                for t4 in range(TC):
                    d_ap = (feats_sp[t4 * 32 + b:t4 * 32 + b + 1, :]
                            .rearrange("p (to tl) -> p to tl", to=LC))
                    s_ap = stage[:, b * T + t4 * TL:b * T + (t4 + 1) * TL]
                    sync.dma_start(d_ap, s_ap).then_inc(sp_sem, 16)
            # chunk-boundary delta copies
            for t4 in range(TC - 1):
                sync.wait_ge(bn_sem, t4 + 1)
                sync.dma_start(
                    delta_all[(t4 + 1) * 32:(t4 + 1) * 32 + BL, 0:LC],
                    delta_all[t4 * 32:t4 * 32 + BL, TL * LC:(TL + 1) * LC],
                ).then_inc(bn2_sem, 16)
            # after scan: delta_510, delta_511 out
            sync.wait_ge(dv_sem, 1)
            sync.dma_start(
                dfin_d[:, :],
                delta_all[96:96 + BL, (TL - 1) * LC:(TL + 1) * LC],
            ).then_inc(out_sem, 16)
            # psiv out after bulk psi
            sync.wait_ge(dv_sem, 2)
            for t4 in range(TC):
                sync.dma_start(psiv_d[t4 * BL:(t4 + 1) * BL, :],
                               psiv[t4 * 32:t4 * 32 + BL, :]
                               ).then_inc(out_sem, 16)

        @block.tensor
        def _(te):
            te.wait_ge(in_sem, 64)
            for b in range(BL):
                te.wait_ge(hs_sems[b % NB], 16 * (b // NB + 1))
                for kc in range(KC):
                    m = te.matmul(
                        psum[b][:, :],
                        wk[:, kc * LC:(kc + 1) * LC],
                        ht[b % NB][:, kc * T:(kc + 1) * T],
                        start=(kc == 0),
                        stop=(kc == KC - 1),
                    )
                    if kc == KC - 1:
                        m.then_inc(pe_sem, 1)

        @block.scalar
        def _(act):
            for b in range(BL):
                act.wait_ge(pe_sem, b + 1)
                act.copy(stage[:, b * T:(b + 1) * T],
                         psum[b][:, :]).then_inc(cp_sem, 1)

        @block.vector
        def _(v):
            # transfeat = trep + feats, sliced; slice 0 before the scan,
            # slices 1..3 interleaved right before the scan needs them
            SL = 32

            def tf_slice(s):
                t0 = s * SL
                in1 = (feats_sp[:, :]
                       .rearrange("p (to tl) -> p tl to", to=LC)
                       [:, t0:t0 + SL, :]
                       .unsqueeze(3).broadcast_to([128, SL, LC, FC]))
                in0 = (trep[:, :].rearrange("p (a f) -> p a f", f=FC)
                       .unsqueeze(1).broadcast_to([128, SL, LC, FC]))
                out4 = (tf[:, t0 * LC * FC:(t0 + SL) * LC * FC]
                        .rearrange("p (tl to f) -> p tl to f", to=LC, f=FC))
                v.tensor_tensor(out4, in0, in1, op=ADD)

            v.wait_ge(sp_sem, 16 * TC * BL)
            tf_slice(0)
            # seed: delta_1 = trans[to',7] + feat_1  -> chunk 0 slot 2
            f1 = (feats_sp[0:BL, :]
                  .rearrange("p (to tl) -> p to tl", to=LC)[:, :, 1:2]
                  .rearrange("p to a -> p (to a)"))
            v.tensor_tensor(delta_all[0:BL, 2 * LC:3 * LC], d7c[:, :], f1,
                            op=ADD)
            v.engine_nop()
            # Viterbi scan: step t reads chunk slot tl, writes slot tl+1
            for t in range(2, T):
                t4, tl = t // TL, t % TL
                base = t4 * 32
                if t4 == 0 and tl % SL == 0 and tl > 0:
                    tf_slice(tl // SL)                   # next transfeat slice
                if t4 > 0 and tl == 0:
                    v.wait_ge(bn2_sem, 16 * t4)          # boundary delta ready
                tf3 = (tf[base:base + BL, tl * LC * FC:(tl + 1) * LC * FC]
                       .rearrange("p (to f) -> p to f", to=LC))
                d3 = (delta_all[base:base + BL, tl * LC:tl * LC + FC]
                      .rearrange("p (a f) -> p a f", a=1)
                      .broadcast_to([BL, LC, FC]))
                s3 = (sc[base:base + BL, :]
                      .rearrange("p (to f) -> p to f", to=LC))
                v.tensor_tensor(s3, tf3, d3, op=ADD)
                r = v.tensor_reduce(
                    delta_all[base:base + BL, (tl + 1) * LC:(tl + 2) * LC],
                    s3, axis=AXX, op=MAX)
                if tl == TL - 1 and t4 < TC - 1:
                    r.then_inc(bn_sem, 1)                # chunk done
                # separate the reduce's tail write from the next TT's
                # head read (same-engine RAW on a pipelined engine)
                v.engine_nop()
            v.engine_nop().then_inc(dv_sem, 1)
            # bulk psi over to' in 0..6, from' in 0..6
            # (delta_all slots 0..127 are exactly delta_{t-1})
            in1 = (delta_all[:, 0:TL * LC]
                   .rearrange("p (tl f) -> p tl f", f=LC)[:, :, 0:FC]
                   .unsqueeze(2).broadcast_to([128, TL, FC, FC]))
            in0 = (trep[:, :].rearrange("p (a f) -> p a f", f=FC)[:, 0:FC, :]
                   .unsqueeze(1).broadcast_to([128, TL, FC, FC]))
            o4 = sca[:, :].rearrange("p (tl to f) -> p tl to f", to=FC, f=FC)
            v.tensor_tensor(o4, in0, in1, op=ADD)
            v.tensor_reduce(mx[:, :], o4, axis=AXX, op=MAX)
            e4 = eq[:, :].rearrange("p (tl to f) -> p tl to f", to=FC, f=FC)
            m4 = (mx[:, :].rearrange("p (tl to) -> p tl to", to=FC)
                  .unsqueeze(3).broadcast_to([128, TL, FC, FC]))
            v.tensor_tensor(e4, o4, m4, op=GE)
            k4 = msk[:, :].rearrange("p (tl to f) -> p tl to f", to=FC, f=FC)
            v.tensor_tensor(k4, e4, rep4(iot, FC, FC), op=MUL)
            v.tensor_reduce(psiv[:, :], k4, axis=AXX, op=MAX)
            v.engine_nop().then_inc(dv_sem, 1)

    return nc


_PROG = None


def _get_prog():
    global _PROG
    if _PROG is None:
        _PROG = build_program()
    return _PROG




def make_in_maps(hidden_states, W, b, transitions):
    hs = np.asarray(hidden_states, np.float32)
    W = np.asarray(W, np.float32)
    bb = np.asarray(b, np.float32)
    trans = np.asarray(transitions, np.float32)

    Wc = W[:, LAB]                                       # [768, 8]
    wk = np.ascontiguousarray(Wc.reshape(KC, 128, LC).transpose(1, 0, 2)
                              ).reshape(128, KC * LC)
    tc_ = (trans + bb[:, None])[np.ix_(LAB, list(range(FC)))]  # [8, 7]
    trep = np.ascontiguousarray(
        np.broadcast_to(tc_.reshape(1, LC * FC), (128, LC * FC)))
    iota = np.broadcast_to((FC - np.arange(FC, dtype=np.float32))[None, :],
                           (FC, FC)).reshape(1, FC * FC)
    import ml_dtypes
    iot = np.ascontiguousarray(np.broadcast_to(iota, (128, FC * FC))
                               ).astype(ml_dtypes.bfloat16)
    d7c = np.ascontiguousarray(
        np.broadcast_to(trans[LAB, START][None, :], (BL, LC))).astype(
            np.float32)

    in_maps = []
    for c in range(NC):
        shard = hs[c * BL:(c + 1) * BL]                 # [8, 512, 768]
        hsT = np.ascontiguousarray(shard.transpose(0, 2, 1))  # [8, 768, 512]
        in_maps.append({"hsT": hsT, "wk": wk, "trep": trep, "iot": iot,
                        "d7c": d7c})
    return in_maps


def decode_core(psiv, dfin, transitions):
    """psiv [32,896] f32, dfin [8,16] f32 -> path [8,512] int32."""
    lab = np.array(LAB, np.int32)
    psi = (FC - psiv.reshape(TC, BL, TL, FC).transpose(1, 0, 2, 3)
           .reshape(BL, T, FC)).astype(np.int32)     # [b, t, to'], t>=2
    d510 = dfin[:, 0:LC]
    d511 = dfin[:, LC:2 * LC]
    p = np.empty((BL, T), np.int32)                  # compact indices
    pf = np.empty((BL, T), np.int32)                 # full labels
    p[:, T - 1] = np.argmax(d511, axis=1)
    pf[:, T - 1] = lab[p[:, T - 1]]
    # psi[511] host-side: argmax over from' 0..6 of trans[to,f]+delta_510[f]
    tr = np.asarray(transitions, np.float32)
    sc511 = tr[lab][:, 0:FC][None] + d510[:, None, 0:FC]   # [b, to', f']
    psi511 = np.argmax(sc511, axis=-1).astype(np.int32)    # [b, to']
    rows = np.arange(BL)
    p[:, T - 2] = psi511[rows, p[:, T - 1]]
    pf[:, T - 2] = p[:, T - 2]                       # from' == full label
    # device psi for t = 510..2  (psi[t] maps path[t] -> path[t-1])
    for t in range(T - 2, 1, -1):
        p[:, t - 1] = psi[rows, t, p[:, t]]          # path[t] in 0..6
        pf[:, t - 1] = p[:, t - 1]
    pf[:, 0] = START
    return pf


def kernel(hidden_states, W, b, transitions):
    in_maps = make_in_maps(hidden_states, W, b, transitions)
    nc = _get_prog()
    res = run_bass_kernel_spmd(nc, in_maps, list(range(NC))).results
    path = np.empty((B, T), np.int32)
    for c in range(NC):
        path[c * BL:(c + 1) * BL] = decode_core(
            res[c]["psiv"], res[c]["dfin"], transitions)
    return path



# revision 16
# speedup vs baseline: 2.3715x; 2.3715x over previous
"""BERT-CRF NER Viterbi decode kernel for Trainium2 (8 NeuronCores), v2.

Strategy (data-parallel over batch, 8 rows/core), raw Bass:
  - host: shard hidden_states [64,512,768] -> 8 x [8,512,768], pre-transpose
    to [8,768,512] and cast to bf16 (halves the dominant HBM read; validated
    ~1e-4..3e-3 path mismatch, far under the 2e-2 gate). W compact+bf16.
  - device (per core):
      feats = W.T @ hsT per batch row -> PSUM [8,512] (6 K-chunks, bf16 PE)
      ACT copies PSUM->SBUF stage; DMA spreads feats to a chunked layout
        feat_sp[p = c*8+b, (to,t_local)]  (C=16 time-chunks of S=32 steps)
      A_t[to,k] = trans[to,k]+bias[to]+feat_t[to] built in one bulk DVE op;
        chunk-0 slots t=0,1 overwritten with the tropical identity (the
        uniform recurrence delta_t = A_t (x) delta_{t-1} starts at t=2 with
        carry delta_1).
      Tree-compose (max,+) matrix products per chunk: pairs -> quads ->
        octs -> 16s -> chunk product E (bulk DVE ops, all chunks in
        parallel across partitions).
      Carry chain: gather E to [b, c] layout, 15 serial matrix-vector
        steps D_{c+1} = E_c (x) D_c, scatter carries back to chunk rows.
      Phase 3 re-scan per chunk: 16 serial vector steps over pair matrices
        (odd positions) + one bulk op over A (even positions) -> delta_t
        for all t, DMA'd out (f32).
      dfin: delta_511 over all 8 reachable labels (incl STOP) for the
        final argmax.
  - host: psi + backtrace from delta (identical argmax semantics to the
    reference; restricted to from-labels 0..6 which provably always win).
"""

import numpy as np
from contextlib import ExitStack

import concourse.bass as bass
from concourse import mybir
from concourse.bass_utils import run_bass_kernel_spmd

B, T, H, L = 64, 512, 768, 9
NC = 8              # cores
BL = B // NC        # batch rows per core = 8
KC = H // 128       # 6 contraction chunks
C = 16              # time chunks per sequence
S = T // C          # 32 steps per chunk
NP = S // 2         # 16 pairs per chunk
START = 7
NEG = -10000.0

F32 = mybir.dt.float32
BF16 = mybir.dt.bfloat16
ADD = mybir.AluOpType.add
MAX = mybir.AluOpType.max
AXX = mybir.AxisListType.X

LC = 8          # compact 'to' labels: (0..6, 8); START row dropped
FC = 7          # compact 'from' labels: 0..6
LAB = [0, 1, 2, 3, 4, 5, 6, 8]


def build_program():
    nc = bass.Bass("TRN2", target_bir_lowering=False,
                   detect_race_conditions=False)

    hsT_d = nc.dram_tensor("hsT", [BL, H, T], BF16, kind="ExternalInput")
    wk_d = nc.dram_tensor("wk", [128, KC * LC], BF16, kind="ExternalInput")
    trep7_d = nc.dram_tensor("trep7", [128, FC * FC], F32,
                             kind="ExternalInput")
    trep8_d = nc.dram_tensor("trep8", [128, LC * FC], F32,
                             kind="ExternalInput")
    d7c_d = nc.dram_tensor("d7c", [BL, FC], F32, kind="ExternalInput")
    ident_d = nc.dram_tensor("ident", [BL, 2 * FC * FC], F32,
                             kind="ExternalInput")
    # bounce buffers for cross-partition regroups
    eg_d = nc.dram_tensor("egb", [128, FC * FC], F32, kind="Internal")
    dg_d = nc.dram_tensor("dgb", [128, FC], F32, kind="Internal")
    st_d = nc.dram_tensor("stb", [128, LC * S], F32, kind="Internal")
    ddel_d = nc.dram_tensor("ddel", [128, S * FC], F32,
                            kind="ExternalOutput")
    dfin_d = nc.dram_tensor("dfin", [BL, LC], F32, kind="ExternalOutput")
    dbg_fsp = nc.dram_tensor("dbg_fsp", [128, LC * S], F32,
                             kind="ExternalOutput")
    dbg_ee = nc.dram_tensor("dbg_ee", [128, FC * FC], F32,
                            kind="ExternalOutput")
    dbg_eg = nc.dram_tensor("dbg_eg", [BL, C * FC * FC], F32,
                            kind="ExternalOutput")
    dbg_dg = nc.dram_tensor("dbg_dg", [BL, C * FC], F32,
                            kind="ExternalOutput")

    with ExitStack() as ctx:
        def sb(name, shape, dt=F32):
            return ctx.enter_context(nc.sbuf_tensor(name, shape, dt))
        wk = sb("wk_sb", [128, KC * LC], BF16)
        trep7 = sb("trep7_sb", [128, FC * FC])
        trep8 = sb("trep8_sb", [128, LC * FC])
        d7c = sb("d7c_sb", [BL, FC])
        ident = sb("ident_sb", [BL, 2 * FC * FC])
        ht = [sb(f"ht{i}", [128, KC * T], BF16) for i in range(BL)]
        stage = sb("stage", [LC, BL * T])
        feat_sp = sb("feat_sp", [128, LC * S])
        A = sb("A_sb", [128, S * FC * FC])
        scw = sb("scw", [128, NP * FC * FC * FC])     # compose scratch
        Bp = sb("Bp", [128, NP * FC * FC])            # pair products
        T2 = sb("T2", [128, 8 * FC * FC])
        T3 = sb("T3", [128, 4 * FC * FC])
        T4 = sb("T4", [128, 2 * FC * FC])
        Ee = sb("Ee", [128, FC * FC])                 # chunk product
        Eg = sb("Eg", [BL, C * FC * FC])              # gathered [b, c]
        Dg = sb("Dg", [BL, C * FC])                   # carries [b, c]
        sc2 = sb("sc2", [BL, FC * FC])
        delta = sb("delta", [128, (S + 1) * FC])      # slot i = local i-1
        sc8 = sb("sc8", [128, LC * FC])
        d8a = sb("d8a", [128, LC])
        dfin = sb("dfin_sb", [128, LC])
        psum = [ctx.enter_context(nc.psum_tensor(f"psum{b}", [LC, T], F32))
                for b in range(BL)]

        in_sem = ctx.enter_context(nc.semaphore("in_sem"))
        hs_sems = [ctx.enter_context(nc.semaphore(f"hs_sem{i}"))
                   for i in range(BL)]
        pe_sem = ctx.enter_context(nc.semaphore("pe_sem"))
        cp_sem = ctx.enter_context(nc.semaphore("cp_sem"))
        sp_sem = ctx.enter_context(nc.semaphore("sp_sem"))
        ev_sem = ctx.enter_context(nc.semaphore("ev_sem"))
        g_sem = ctx.enter_context(nc.semaphore("g_sem"))
        p2_sem = ctx.enter_context(nc.semaphore("p2_sem"))
        sct_sem = ctx.enter_context(nc.semaphore("sct_sem"))
        dv_sem = ctx.enter_context(nc.semaphore("dv_sem"))
        out_sem = ctx.enter_context(nc.semaphore("out_sem"))
        block = ctx.enter_context(nc.Block())

        @block.sync
        def _(sync):
            sync.dma_start(wk[:, :], wk_d[:, :]).then_inc(in_sem, 16)
            sync.dma_start(trep7[:, :], trep7_d[:, :]).then_inc(in_sem, 16)
            sync.dma_start(trep8[:, :], trep8_d[:, :]).then_inc(in_sem, 16)
            sync.dma_start(d7c[:, :], d7c_d[:, :]).then_inc(in_sem, 16)
            sync.dma_start(ident[:, :], ident_d[:, :]).then_inc(in_sem, 16)
            for b in range(0, 4):
                src = hsT_d[b, :, :].rearrange("(kc p) t -> p kc t", p=128)
                dst = ht[b][:, :].rearrange("p (kc t) -> p kc t", kc=KC)
                sync.dma_start(dst, src).then_inc(hs_sems[b], 16)
            # spread hop 2: st_d (already in (c,b)-row order) -> feat_sp
            sync.wait_ge(sp_sem, 16 * BL)
            sync.dma_start(feat_sp[:, :], st_d[:, :]).then_inc(sp_sem, 16)
            # gather chunk products E[(c,b)] -> Eg[b, (c,...)] via DRAM
            sync.wait_ge(ev_sem, 1)
            sync.dma_start(eg_d[:, :], Ee[:, :]).then_inc(g_sem, 16)
            sync.wait_ge(g_sem, 16)
            sync.dma_start(
                Eg[:, :].rearrange("b (c f) -> b c f", f=FC * FC),
                eg_d[:, :].rearrange("(c b) f -> b c f", b=BL),
            ).then_inc(g_sem, 16)
            # scatter carries Dg[b, c] -> delta[(c,b), slot 0] via DRAM
            sync.wait_ge(p2_sem, 1)
            sync.dma_start(
                dg_d[:, :].rearrange("(c b) f -> b c f", b=BL),
                Dg[:, :].rearrange("b (c f) -> b c f", f=FC),
            ).then_inc(sct_sem, 16)
            sync.wait_ge(sct_sem, 16)
            sync.dma_start(delta[:, 0:FC], dg_d[:, :]).then_inc(sct_sem, 16)
            # outputs
            sync.wait_ge(dv_sem, 1)
            sync.dma_start(ddel_d[:, :],
                           delta[:, FC:(S + 1) * FC]).then_inc(out_sem, 16)
            sync.dma_start(dfin_d[:, :],
                           dfin[120:128, :]).then_inc(out_sem, 16)
            sync.dma_start(dbg_fsp[:, :], feat_sp[:, :]).then_inc(out_sem, 16)
            sync.dma_start(dbg_ee[:, :], Ee[:, :]).then_inc(out_sem, 16)
            sync.dma_start(dbg_eg[:, :], Eg[:, :]).then_inc(out_sem, 16)
            sync.dma_start(dbg_dg[:, :], Dg[:, :]).then_inc(out_sem, 16)

        @block.scalar
        def _(act):
            for b in range(4, BL):
                src = hsT_d[b, :, :].rearrange("(kc p) t -> p kc t", p=128)
                dst = ht[b][:, :].rearrange("p (kc t) -> p kc t", kc=KC)
                act.dma_start(dst, src).then_inc(hs_sems[b], 16)
            for b in range(BL):
                act.wait_ge(pe_sem, b + 1)
                act.copy(stage[:, b * T:(b + 1) * T],
                         psum[b][:, :]).then_inc(cp_sem, 1)
                # barrier: wait for the copy's own sem so its tail writes
                # land before the spread DMA reads stage
                act.wait_ge(cp_sem, b + 1)
                # spread hop 1: stage[to, b-block] -> st_d rows {c*8+b},
                # reordered (to, c, t) on the DRAM side
                dst = (st_d[:, :]
                       .rearrange("(c b) (to t) -> b to c t", b=BL, t=S)
                       [b])
                src = (stage[:, b * T:(b + 1) * T]
                       .rearrange("to (c t) -> to c t", t=S))
                act.dma_start(dst, src).then_inc(sp_sem, 16)

        @block.tensor
        def _(te):
            te.wait_ge(in_sem, 80)
            for b in range(BL):
                te.wait_ge(hs_sems[b], 16)
                for kc in range(KC):
                    m = te.matmul(
                        psum[b][:, :],
                        wk[:, kc * LC:(kc + 1) * LC],
                        ht[b][:, kc * T:(kc + 1) * T],
                        start=(kc == 0),
                        stop=(kc == KC - 1),
                    )
                    if kc == KC - 1:
                        m.then_inc(pe_sem, 1)

        @block.vector
        def _(v):
            Av = A[:, :].rearrange("p (t to k) -> p t to k", to=FC, k=FC)
            # A as [t, k, to] view: reading (f=inner, k=outer) per t
            Aw = A[:, :].rearrange("p (t to k) -> p t k to", to=FC, k=FC)
            Bv = Bp[:, :].rearrange("p (j to f) -> p j to f", to=FC, f=FC)
            dlt = delta[:, :].rearrange("p (s f) -> p s f", f=FC)

            v.wait_ge(in_sem, 80)
            v.wait_ge(sp_sem, 16 * BL + 16)
            # A[t,to,k] = trep7[to,k] + feat[to,t]
            fv = (feat_sp[:, :].rearrange("p (to t) -> p t to", to=LC)
                  [:, :, 0:FC].unsqueeze(3).broadcast_to([128, S, FC, FC]))
            tv = (trep7[:, :].rearrange("p (to k) -> p to k", k=FC)
                  .unsqueeze(1).broadcast_to([128, S, FC, FC]))
            v.tensor_tensor(Av, tv, fv, op=ADD)
            v.engine_nop()
            # chunk-0 slots t=0,1 := tropical identity
            v.tensor_scalar_add(A[0:BL, 0:2 * FC * FC], ident[:, :], 0.0)
            # seed carry: D_0 = delta_1 = trans[f,START]+bias[f]+feat_1[f]
            f1 = (feat_sp[0:BL, :].rearrange("p (to t) -> p to t", to=LC)
                  [:, 0:FC, 1:2].rearrange("p f a -> p (f a)"))
            v.tensor_tensor(Dg[:, 0:FC], d7c[:, :], f1, op=ADD)
            v.engine_nop()

            def compose(dst, src, n, src_j):
                """dst[j] = src[2j+1] (x) src[2j] for j in 0..n.

                TT is limited to 3 free dims, so split by 'to' (in0 has
                f-stride 0 which blocks a (to,f) merge); the reduce merges
                (to,f) legally and runs once.
                """
                sv = src.rearrange("p (j to k) -> p j to k", to=FC, k=FC)
                sw = src.rearrange("p (j to k) -> p j k to", to=FC, k=FC)
                lo = sw[:, 0:2 * n:2]                  # [p, j, f, k]
                ov = (scw[:, 0:n * FC * FC * FC]
                      .rearrange("p (j to f k) -> p to j f k",
                                 to=FC, f=FC, k=FC))
                for to in range(FC):
                    hi = (sv[:, 1:2 * n:2, to, :].unsqueeze(2)
                          .broadcast_to([128, n, FC, FC]))
                    v.tensor_tensor(ov[:, to], hi, lo, op=ADD)
                o3 = (scw[:, 0:n * FC * FC * FC]
                      .rearrange("p (j tof k) -> p j tof k",
                                 tof=FC * FC, k=FC))
                d2 = dst.rearrange("p (j tof) -> p j tof", tof=FC * FC)
                v.tensor_reduce(d2, o3, axis=AXX, op=MAX)
                v.engine_nop()
                v.engine_nop()

            compose(Bp[:, :], A[:, :], NP, S)        # pairs
            compose(T2[:, :], Bp[:, :], 8, NP)
            compose(T3[:, :], T2[:, :], 4, 8)
            compose(T4[:, :], T3[:, :], 2, 4)
            compose(Ee[:, :], T4[:, :], 1, 2)
            v.engine_nop().then_inc(ev_sem, 1)

            # phase 2: carries D_{c+1} = E_c (x) D_c  (b-partition layout)
            v.wait_ge(g_sem, 32)
            egv = Eg[:, :].rearrange("p (c to k) -> p c to k", to=FC, k=FC)
            s2 = sc2[:, :].rearrange("p (to k) -> p to k", k=FC)
            for c in range(C - 1):
                din = (Dg[:, c * FC:(c + 1) * FC]
                       .rearrange("p (a k) -> p a k", a=1)
                       .broadcast_to([BL, FC, FC]))
                v.tensor_tensor(s2, egv[:, c], din, op=ADD)
                v.tensor_reduce(Dg[:, (c + 1) * FC:(c + 2) * FC], s2,
                                axis=AXX, op=MAX)
                v.engine_nop()
                v.engine_nop()
            v.engine_nop().then_inc(p2_sem, 1)

            # phase 3: re-scan. odd locals via pairs (serial), evens bulk.
            v.wait_ge(sct_sem, 32)
            s3 = scw[:, 0:FC * FC].rearrange("p (to k) -> p to k", k=FC)
            for j in range(NP):
                din = (delta[:, 2 * j * FC:(2 * j + 1) * FC]
                       .rearrange("p (a k) -> p a k", a=1)
                       .broadcast_to([128, FC, FC]))
                v.tensor_tensor(s3, Bv[:, j], din, op=ADD)
                v.tensor_reduce(delta[:, (2 * j + 2) * FC:(2 * j + 3) * FC],
                                s3, axis=AXX, op=MAX)
                v.engine_nop()
                v.engine_nop()
            # evens: local_{2j} = A_{2j} (x) local_{2j-1} for all j at once
            ae = (Av[:, 0:S:2, :, :])                          # [p,16,7,7]
            de = (dlt[:, 0:S:2, :].unsqueeze(2)
                  .broadcast_to([128, NP, FC, FC]))
            oe = (scw[:, 0:NP * FC * FC]
                  .rearrange("p (j to k) -> p j to k", to=FC, k=FC))
            v.tensor_tensor(oe, ae, de, op=ADD)
            v.tensor_reduce(dlt[:, 1:S:2, :], oe, axis=AXX, op=MAX)
            v.engine_nop()
            # dfin: delta_511 over all 8 labels = trep8 (x) d510 + feat_511
            # (computed on partitions 96:128 for alignment; 96..120 garbage)
            t8 = trep8[96:128, :].rearrange("p (to k) -> p to k", k=FC)
            d510 = (delta[96:128, (S - 1) * FC:S * FC]
                    .rearrange("p (a k) -> p a k", a=1)
                    .broadcast_to([32, LC, FC]))
            s8 = sc8[96:128, :].rearrange("p (to k) -> p to k", k=FC)
            v.tensor_tensor(s8, t8, d510, op=ADD)
            v.tensor_reduce(d8a[96:128, :], s8, axis=AXX, op=MAX)
            v.engine_nop()
            f511 = (feat_sp[96:128, :].rearrange("p (to t) -> p to t", to=LC)
                    [:, :, S - 1:S].rearrange("p to a -> p (to a)"))
            v.tensor_tensor(dfin[96:128, :], d8a[96:128, :], f511, op=ADD)
            v.engine_nop().then_inc(dv_sem, 1)

    return nc


_PROG = None


def _get_prog():
    global _PROG
    if _PROG is None:
        _PROG = build_program()
    return _PROG


def make_in_maps(hidden_states, W, b, transitions):
    import ml_dtypes
    hs = np.asarray(hidden_states, np.float32)
    W = np.asarray(W, np.float32)
    bb = np.asarray(b, np.float32)
    trans = np.asarray(transitions, np.float32)

    Wc = W[:, LAB]                                       # [768, 8]
    wk = np.ascontiguousarray(Wc.reshape(KC, 128, LC).transpose(1, 0, 2)
                              ).reshape(128, KC * LC).astype(ml_dtypes.bfloat16)
    t7 = (trans + bb[:, None])[0:FC, 0:FC]               # [7, 7]
    trep7 = np.ascontiguousarray(
        np.broadcast_to(t7.reshape(1, FC * FC), (128, FC * FC))).astype(
            np.float32)
    t8 = (trans + bb[:, None])[np.ix_(LAB, list(range(FC)))]   # [8, 7]
    trep8 = np.ascontiguousarray(
        np.broadcast_to(t8.reshape(1, LC * FC), (128, LC * FC))).astype(
            np.float32)
    d7c = np.ascontiguousarray(
        np.broadcast_to((trans[0:FC, START] + bb[0:FC])[None, :],
                        (BL, FC))).astype(np.float32)
    idm = np.where(np.eye(FC, dtype=bool), 0.0, NEG).astype(np.float32)
    ident = np.ascontiguousarray(
        np.broadcast_to(np.concatenate([idm.reshape(-1)] * 2)[None, :],
                        (BL, 2 * FC * FC))).astype(np.float32)

    in_maps = []
    for c in range(NC):
        shard = hs[c * BL:(c + 1) * BL]                 # [8, 512, 768]
        hsT = np.ascontiguousarray(shard.transpose(0, 2, 1)).astype(
            ml_dtypes.bfloat16)                         # [8, 768, 512]
        in_maps.append({"hsT": hsT, "wk": wk, "trep7": trep7,
                        "trep8": trep8, "d7c": d7c, "ident": ident})
    return in_maps


def decode(ddel_list, dfin_list, transitions):
    """ddel [128, 224] f32 per core, dfin [8,8] f32 per core -> [64,512]."""
    trans = np.asarray(transitions, np.float32)
    lab = np.array(LAB, np.int64)
    delta = np.empty((B, T, FC), np.float32)
    d8 = np.empty((B, LC), np.float32)
    for c in range(NC):
        dd = ddel_list[c].reshape(C, BL, S, FC)          # [(c,b), j, f]
        delta[c * BL:(c + 1) * BL] = (dd.transpose(1, 0, 2, 3)
                                      .reshape(BL, T, FC))
        d8[c * BL:(c + 1) * BL] = dfin_list[c]
    path = np.empty((B, T), np.int32)
    cur = lab[np.argmax(d8, axis=1)]                     # labels, may be 8
    path[:, T - 1] = cur
    rows = np.arange(B)
    for t in range(T - 1, 1, -1):
        cur = np.argmax(trans[cur, 0:FC] + delta[:, t - 1, :], axis=1)
        path[:, t - 1] = cur
    path[:, 0] = START
    return path


def kernel(hidden_states, W, b, transitions):
    in_maps = make_in_maps(hidden_states, W, b, transitions)
    nc = _get_prog()
    res = run_bass_kernel_spmd(nc, in_maps, list(range(NC))).results
    return decode([res[c]["ddel"] for c in range(NC)],
                  [res[c]["dfin"] for c in range(NC)], transitions)
